# revision 55
# baseline (speedup 1.0000x reference)
"""Multi-head attention (B=4, S=2048, D=1024, H=16) on 8 trn2 NeuronCores.

Sharding: core c -> (batch b = c//2, head-group hg = c%2 of 8 heads).
Each core computes q/k/v projections for its 8 heads, attention, and a
partial output projection (its heads' contribution). Host sums the two
partials per batch and adds b_O.

Single interleaved pipeline (the ACT engine's exp throughput, ~1.1us per
[128,1024] tile x 256 tiles = ~285us, is the hard floor; everything else
hides behind it):
  - minimal lead-in: kT[hp0], qT[hp0][qc0], vhat[st0]
  - attention blocks (hp, qc) x16 kt: scoresT pair (row-tiled K=64
    concurrent matmuls), ACT exp -> PT, PV accumulate (ones column in
    vhat makes softmax Z fall out of the PV matmul)
  - all remaining projection work + normalization tails + output
    projection run as backlog pieces popped one per kt iteration
  - normalization: stage psum->SBUF, sync-DMA the Z row to partition 0,
    reciprocal_approx_fast, gpsimd partition_broadcast (no PE/PSUM),
    DVE multiply
  - output projection: PSUM-chained over all 4 head-pairs during hp3
"""
import sys

if '/opt/trn_rl_repo' not in sys.path:
    sys.path.insert(0, '/opt/trn_rl_repo')

from collections import deque
from contextlib import ExitStack

import ml_dtypes
import numpy as np

import concourse.bass as bass
import concourse.tile as tile
from concourse import bacc, mybir
from concourse.bass_utils import run_bass_kernel_spmd

N_CORES = 8
B, S, D = 4, 2048, 1024
H = 16
DH = 64                 # head dim
HC = 8                  # heads per core
C = HC * DH             # per-core projection width = 512
SH = S // 2             # S half = 1024
F32 = mybir.dt.float32
F32R = mybir.dt.float32r
BF16 = mybir.dt.bfloat16

NKT = S // 128          # 16 s-tiles of 128
NM = C // 128           # 4 c-tiles (head pairs)
NDK = D // 128          # 8 contraction tiles for projections
SCALE = 1.0 / np.sqrt(DH)

PROJ_DT = BF16
QK_DT = BF16
PV_DT = BF16
OUT_DT = BF16


def round_fp32r(x):
    b = np.ascontiguousarray(x, dtype=np.float32).view(np.uint32)
    b = (b + 0x800) & np.uint32(0xFFFFF000)
    return b.view(np.float32)


def prep(x, dt):
    if dt == BF16:
        return np.ascontiguousarray(x).astype(ml_dtypes.bfloat16)
    return round_fp32r(x)


def build():
    nc = bacc.Bacc("TRN2", target_bir_lowering=False, debug=False,
                   num_devices=N_CORES)
    XqT = nc.dram_tensor("XqT", [D, S], PROJ_DT, kind="ExternalInput").ap()
    XkT = nc.dram_tensor("XkT", [D, S], PROJ_DT, kind="ExternalInput").ap()
    XvT = nc.dram_tensor("XvT", [D, S], PROJ_DT, kind="ExternalInput").ap()
    Wq = nc.dram_tensor("Wq", [D, C], PROJ_DT, kind="ExternalInput").ap()
    Wk = nc.dram_tensor("Wk", [D, C], PROJ_DT, kind="ExternalInput").ap()
    Wv = nc.dram_tensor("Wv", [D, C], PROJ_DT, kind="ExternalInput").ap()
    Wo = nc.dram_tensor("Wo", [C, D], OUT_DT, kind="ExternalInput").ap()
    bq = nc.dram_tensor("bq", [C], F32, kind="ExternalInput").ap()
    bk = nc.dram_tensor("bk", [C], F32, kind="ExternalInput").ap()
    bv = nc.dram_tensor("bv", [C], F32, kind="ExternalInput").ap()
    OP = nc.dram_tensor("OP", [S, D], F32, kind="ExternalOutput").ap()

    with tile.TileContext(nc) as tc:
        _build_body(nc, tc, XqT, XkT, XvT, Wq, Wk, Wv, Wo, bq, bk, bv, OP)
    nc.compile()
    return nc


def _build_body(nc, tc, XqT, XkT, XvT, Wq, Wk, Wv, Wo, bq, bk, bv, OP):
    with ExitStack() as stack:
        ep = stack.enter_context
        consts = ep(tc.tile_pool(name="consts", bufs=1))
        wkp = ep(tc.tile_pool(name="wk", bufs=NDK))
        wqp = ep(tc.tile_pool(name="wq", bufs=NDK))
        wvp = ep(tc.tile_pool(name="wv", bufs=NDK))
        wop = ep(tc.tile_pool(name="wo", bufs=NM))
        xkp = ep(tc.tile_pool(name="xk", bufs=2 * NDK))   # XkT halves
        xqp = ep(tc.tile_pool(name="xq", bufs=NDK))       # XqT half 0
        xsp = ep(tc.tile_pool(name="xs", bufs=2 * NDK))   # XvT, then XqT h1
        kqp = ep(tc.tile_pool(name="kq", bufs=4))         # rolling kT/qT
        vhp = ep(tc.tile_pool(name="vh", bufs=NKT))
        aop = ep(tc.tile_pool(name="aout", bufs=NM))
        ptp = ep(tc.tile_pool(name="pt", bufs=6))
        stgp = ep(tc.tile_pool(name="stg", bufs=4))
        nrmp = ep(tc.tile_pool(name="nrm", bufs=4))
        bcp = ep(tc.tile_pool(name="bc", bufs=2))
        osp = ep(tc.tile_pool(name="ostg", bufs=2))
        sp = ep(tc.tile_pool(name="sps", bufs=2, space="PSUM"))
        pvp = ep(tc.tile_pool(name="pv", bufs=2, space="PSUM"))
        opp = ep(tc.tile_pool(name="op", bufs=2, space="PSUM"))

        # ---------------- constants ----------------
        ones_f32 = consts.tile([128, 1], F32)
        nc.vector.memset(ones_f32, 1.0)

        bias_t = consts.tile([128, 2 * NM], F32)
        for i, b_ in enumerate((bq, bk)):
            nc.sync.dma_start(
                out=bias_t[:, i * NM:(i + 1) * NM],
                in_=b_.rearrange("(m p) -> p m", p=128))
        bvb = consts.tile([128, C], F32)
        nc.gpsimd.dma_start(
            out=bvb,
            in_=bass.AP(tensor=bv.tensor, offset=0, ap=[[0, 128], [1, C]]))

        # ---------------- weight + X DMAs, critical-first ----------------
        # Half-tiles [128, 1024] keyed (half, kt), ordered so the lead-in
        # pieces' data (Wk+XkT-h0, Wq+XqT-h0, Wv+XvT-h0) arrives first.
        wk_t, wq_t, wv_t, wo_t = [None] * NDK, [None] * NDK, [None] * NDK, []
        xk_t, xq_t, xv_t = {}, {}, {}

        def xdma(pool, dst, X, half, tag):
            for kt in range(NDK):
                t = pool.tile([128, SH], PROJ_DT, tag=tag,
                              name=f"{tag}{half}_{kt}")
                nc.sync.dma_start(
                    out=t, in_=X[kt * 128:(kt + 1) * 128,
                                 half * SH:(half + 1) * SH])
                dst[(half, kt)] = t

        for kt in range(NDK):
            w = wkp.tile([128, C], PROJ_DT, tag="wk", name=f"wk{kt}")
            nc.sync.dma_start(out=w, in_=Wk[kt * 128:(kt + 1) * 128, :])
            wk_t[kt] = w
        for kt in range(NDK):
            w = wqp.tile([128, C], PROJ_DT, tag="wq", name=f"wq{kt}")
            nc.sync.dma_start(out=w, in_=Wq[kt * 128:(kt + 1) * 128, :])
            wq_t[kt] = w
        # interleave XkT-h0/XqT-h0 tiles so k000 and q000 both stream in
        # progressively (avoids a 4.5us PE gap while q000 waits)
        for kt in range(NDK):
            t = xkp.tile([128, SH], PROJ_DT, tag="xk", name=f"xk0_{kt}")
            nc.sync.dma_start(out=t, in_=XkT[kt * 128:(kt + 1) * 128, 0:SH])
            xk_t[(0, kt)] = t
            t = xqp.tile([128, SH], PROJ_DT, tag="xq", name=f"xq0_{kt}")
            nc.sync.dma_start(out=t, in_=XqT[kt * 128:(kt + 1) * 128, 0:SH])
            xq_t[(0, kt)] = t
        for kt in range(NDK):
            w = wvp.tile([128, C], PROJ_DT, tag="wv", name=f"wv{kt}")
            nc.sync.dma_start(out=w, in_=Wv[kt * 128:(kt + 1) * 128, :])
            wv_t[kt] = w
        xdma(xsp, xv_t, XvT, 0, "xs")
        xdma(xkp, xk_t, XkT, 1, "xk")
        xdma(xsp, xv_t, XvT, 1, "xs")
        for m in range(NM):
            w = wop.tile([128, D], OUT_DT, tag="wo", name=f"wo{m}")
            nc.sync.dma_start(out=w, in_=Wo[m * 128:(m + 1) * 128, :])
            wo_t.append(w)

        # ---------------- rolling kT/qT tiles ----------------
        kq_tiles = {}

        def kq_tile(pk, hp):
            key = (pk, hp)
            if key not in kq_tiles:
                kq_tiles[key] = kqp.tile([128, S], QK_DT, tag="kq",
                                         name=f"{pk}T{hp}")
            return kq_tiles[key]

        vhat = [None] * NKT
        attn_outT = {}

        def attn_tile(hp):
            if hp not in attn_outT:
                attn_outT[hp] = aop.tile([128, S], OUT_DT, tag="aout",
                                         name=f"aoutT{hp}")
            return attn_outT[hp]

        # ---------------- backlog piece definitions ----------------
        # Each piece is a closure emitting ~<=1-2us of PE work.  Projection
        # pieces are split in two sub-pieces (4 contraction matmuls each)
        # to keep per-slot PE bursts under the one-iteration sps lookahead.

        def proj_kq_sub(pk, hp, half, sc, phase, ps_box):
            XT = xk_t if pk == 'k' else xq_t
            WT = wk_t if pk == 'k' else wq_t
            bcol = (NM if pk == 'k' else 0) + hp

            def run():
                if phase == 0:
                    ps_box[0] = opp.tile([128, 512], F32, tag="op",
                                        name=f"pj{pk}{hp}_{half}{sc}")
                ps = ps_box[0]
                for kt in range(phase * 4, phase * 4 + 4):
                    nc.tensor.matmul(
                        ps,
                        WT[kt][:, hp * 128:(hp + 1) * 128],
                        XT[(half, kt)][:, sc * 512:(sc + 1) * 512],
                        start=(kt == 0), stop=(kt == NDK - 1))
                if phase == 1:
                    dst = kq_tile(pk, hp)
                    s0 = half * SH + sc * 512
                    with nc.allow_low_precision(reason="proj epilogue"):
                        nc.vector.tensor_add(
                            dst[:, s0:s0 + 512], ps,
                            bias_t[:, bcol:bcol + 1].broadcast_to((128, 512)))
            return run

        def proj_kq_piece(pk, hp, half, sc):
            box = [None]
            return [proj_kq_sub(pk, hp, half, sc, 0, box),
                    proj_kq_sub(pk, hp, half, sc, 1, box)]

        def proj_v_piece(st):
            half, stl = st // 8, st % 8

            def run():
                ps = opp.tile([128, C], F32, tag="op", name=f"pjv{st}")
                for kt in range(NDK):
                    nc.tensor.matmul(
                        ps,
                        xv_t[(half, kt)][:, stl * 128:(stl + 1) * 128],
                        wv_t[kt],
                        start=(kt == 0), stop=(kt == NDK - 1))
                vh = vhp.tile([128, HC, DH + 1], PV_DT, tag="vh",
                              name=f"vhat{st}")
                with nc.allow_low_precision(reason="v epilogue"):
                    nc.vector.tensor_add(
                        vh[:, :, 0:DH],
                        ps.rearrange("p (h d) -> p h d", h=HC),
                        bvb.rearrange("p (h d) -> p h d", h=HC))
                    nc.vector.tensor_copy(
                        vh[:, :, DH], ones_f32.broadcast_to((128, HC)))
                vhat[st] = vh
            return run

        def xq_h1_dma_piece():
            def run():
                for kt in range(NDK):
                    t = xsp.tile([128, SH], PROJ_DT, tag="xs",
                                 name=f"xqh1_{kt}")
                    nc.sync.dma_start(
                        out=t, in_=XqT[kt * 128:(kt + 1) * 128, SH:S])
                    xq_t[(1, kt)] = t
            return run

        # normalization tail for one (hp, qc) block
        def tail_pieces(hp, qc, stA, stB):
            q0 = qc * 512
            sts = (stA, stB)
            zrows = [None, None]
            rzs = [None, None]
            bcs = [None, None]

            def zdma(hh):
                # custom-DVE ops require base partition 0: move the Z row
                # (partition 64 of the staged tile) to a partition-0 tile
                # via the sync DMA path (same as all bulk loads)
                def run():
                    zr = nrmp.tile([1, 512], F32, tag="zr",
                                   name=f"zr{hp}_{qc}_{hh}")
                    nc.sync.dma_start(out=zr, in_=sts[hh][DH:DH + 1, :])
                    zrows[hh] = zr
                return run

            def recip(hh):
                def run():
                    rz = nrmp.tile([1, 512], F32, tag="rz",
                                   name=f"rz{hp}_{qc}_{hh}")
                    nc.vector.reciprocal_approx_fast(out=rz, in_=zrows[hh])
                    rzs[hh] = rz
                    if hp == 0 and qc == 0 and hh == 0:
                        _DEBUG_TILES['rz00A'] = rz
                return run

            def bcast(hh):
                def run():
                    bc = bcp.tile([DH, 512], F32, tag="bc",
                                  name=f"bc{hp}_{qc}_{hh}")
                    nc.gpsimd.partition_broadcast(bc, rzs[hh], channels=DH)
                    bcs[hh] = bc
                return run

            def mul(hh):
                def run():
                    dlo = hh * DH
                    with nc.allow_low_precision(reason="attn_outT"):
                        nc.vector.tensor_mul(
                            attn_tile(hp)[dlo:dlo + DH, q0:q0 + 512],
                            sts[hh][0:DH, :], bcs[hh])
                return run

            return [zdma(0), zdma(1), recip(0), recip(1),
                    bcast(0), bcast(1), mul(0), mul(1)]

        # output projection piece: one st block (128 tokens), both oc halves,
        # PSUM-accumulated over all 4 head-pairs
        def outproj_sub(st, oc):
            def run():
                ps = opp.tile([128, 512], F32, tag="op",
                              name=f"ops{st}_{oc}")
                for hp in range(NM):
                    nc.tensor.matmul(
                        ps,
                        attn_outT[hp][:, st * 128:(st + 1) * 128],
                        wo_t[hp][:, oc * 512:(oc + 1) * 512],
                        start=(hp == 0), stop=(hp == NM - 1))
                ot = osp.tile([128, 512], F32, tag="os", name=f"ot{st}_{oc}")
                nc.vector.tensor_copy(ot, ps)
                nc.sync.dma_start(
                    out=OP[st * 128:(st + 1) * 128, oc * 512:(oc + 1) * 512],
                    in_=ot)
            return run

        # ---------------- static slot schedule ----------------
        # block index b = hp*4 + qc; 16 slots per block (one per kt)
        static_slots = {b: [] for b in range(16)}

        # b0: vhat st1..15, ordered so st(j+1) is emitted at slot j
        # (PV(kt) for st j+1 comes after slot j in program order)
        # b0 pops TWO pieces per kt.  kT0's h1 pieces are placed as late
        # as their scores-consumer allows so their XkT-h1 DMAs have
        # arrived; vhat st_j pieces must precede PV(kt_j).  k001/q001
        # (h0 data, resident early) run in the lead-in instead, filling
        # the PE-idle gap while Wv/XvT stream in (keeps HAM warm).
        v = [proj_v_piece(j) for j in range(2, 16)]
        k010 = proj_kq_piece('k', 0, 1, 0)
        k011 = proj_kq_piece('k', 0, 1, 1)
        static_slots[0] = (
            [v[0], v[1], v[2], v[3], v[4], v[5],
             v[6], v[7], k010[0], k010[1], v[8], v[9], v[10], v[11],
             k011[0], k011[1], v[12], v[13]])

        # Remaining projection pieces with explicit block assignments.
        # Constraints: a piece must be emitted in a block strictly before
        # its consumer block, AND not before the kq ring buffer it reuses
        # (bufs=4: kT2<-kT0 slot, qT2<-qT0, kT3<-kT1, qT3<-qT1) has had its
        # last read emitted (kT0/qT0 read through b=3, kT1/qT1 through b=7).
        # Finer deadlines: kT[hp](half,sc) is first read at block (hp,*)
        # iteration kt = half*8+sc*4, so a piece may pop early IN its
        # consumer hp's first block.  ~2 pieces/block evens out PE load.
        sched = [
            (1, [xq_h1_dma_piece()]),
            (1, proj_kq_piece('q', 0, 1, 0)),   # qc2 of hp0 (b=2)
            (2, proj_kq_piece('q', 0, 1, 1)),   # qc3 of hp0 (b=3)
            (2, proj_kq_piece('k', 1, 0, 0)),
            (3, proj_kq_piece('k', 1, 0, 1)),
            (3, proj_kq_piece('q', 1, 0, 0)),
            (3, proj_kq_piece('k', 1, 1, 0)),   # b4-kt8: after b4's 8 tail
            (4, proj_kq_piece('k', 1, 1, 1)),   # pops it would land too late
            (4, proj_kq_piece('q', 1, 0, 1)),
            (5, proj_kq_piece('q', 1, 1, 0)),
            (5, proj_kq_piece('k', 2, 0, 0)),
            (6, proj_kq_piece('q', 1, 1, 1)),
            (6, proj_kq_piece('k', 2, 0, 1)),
            (7, proj_kq_piece('k', 2, 1, 0)),   # read from b8-kt8
            (7, proj_kq_piece('k', 2, 1, 1)),
            (7, proj_kq_piece('q', 2, 0, 0)),
            (8, proj_kq_piece('q', 2, 0, 1)),
            (8, proj_kq_piece('k', 3, 0, 0)),
            (9, proj_kq_piece('q', 2, 1, 0)),
            (9, proj_kq_piece('k', 3, 0, 1)),
            (10, proj_kq_piece('q', 2, 1, 1)),
            (10, proj_kq_piece('k', 3, 1, 0)),  # read from b12-kt8
            (11, proj_kq_piece('k', 3, 1, 1)),
            (11, proj_kq_piece('q', 3, 0, 0)),
            (12, proj_kq_piece('q', 3, 0, 1)),
            (12, proj_kq_piece('q', 3, 1, 0)),
            (12, proj_kq_piece('q', 3, 1, 1)),
        ]
        for bidx, piece in sched:
            static_slots[bidx].extend(piece)

        # ---------------- lead-in ----------------
        for sub in proj_kq_piece('k', 0, 0, 0):
            sub()
        for sub in proj_kq_piece('q', 0, 0, 0):
            sub()
        for sub in proj_kq_piece('k', 0, 0, 1):
            sub()
        for sub in proj_kq_piece('q', 0, 0, 1):
            sub()
        proj_v_piece(0)()
        proj_v_piece(1)()

        # ---------------- main attention loop ----------------
        slot_q = deque()
        for hp in range(NM):
            kT = kq_tile('k', hp)
            qT = kq_tile('q', hp)
            for qc in range(4):
                b = hp * 4 + qc
                slot_q.extend(static_slots[b])
                q0 = qc * 512
                pvA = pvp.tile([DH + 1, 512], F32, tag="pv",
                               name=f"pvA{hp}_{qc}")
                pvB = pvp.tile([DH + 1, 512], F32, tag="pv",
                               name=f"pvB{hp}_{qc}")
                for kt in range(NKT):
                    sps = sp.tile([128, 1024], F32, tag="sps")
                    for hh in range(2):
                        dlo = hh * DH
                        nc.tensor.matmul(
                            sps[:, hh * 512:(hh + 1) * 512],
                            kT[dlo:dlo + DH, kt * 128:(kt + 1) * 128],
                            qT[dlo:dlo + DH, q0:q0 + 512],
                            start=True, stop=True)
                    pt = ptp.tile([128, 1024], PV_DT, tag="pt")
                    nc.scalar.activation(
                        out=pt, in_=sps,
                        func=mybir.ActivationFunctionType.Exp,
                        scale=float(SCALE))
                    nc.tensor.matmul(
                        pvA, vhat[kt][:, 2 * hp, :], pt[:, 0:512],
                        start=(kt == 0), stop=(kt == NKT - 1))
                    nc.tensor.matmul(
                        pvB, vhat[kt][:, 2 * hp + 1, :], pt[:, 512:1024],
                        start=(kt == 0), stop=(kt == NKT - 1))
                    for _ in range(2 if b == 0 else 1):
                        if slot_q:
                            slot_q.popleft()()
                # block tail: stage to SBUF (frees the PV psum quickly)
                stA = stgp.tile([DH + 1, 512], F32, tag="stg",
                                name=f"stgA{hp}_{qc}")
                nc.vector.tensor_copy(stA, pvA)
                stB = stgp.tile([DH + 1, 512], F32, tag="stg",
                                name=f"stgB{hp}_{qc}")
                nc.vector.tensor_copy(stB, pvB)
                if hp == 0 and qc == 0:
                    _DEBUG_TILES['st00A'] = stA
                # normalization chain deferred into the next block's slots
                # (spreads the zdma->recip->bcast->mul latency chain)
                slot_q.extend(tail_pieces(hp, qc, stA, stB))
                if hp == NM - 1:
                    # output projection for THIS qc of hp3 (slots of the
                    # next block; qc3's pieces drain in the flush)
                    for st in range(qc * 4, qc * 4 + 4):
                        slot_q.append(outproj_sub(st, 0))
                        slot_q.append(outproj_sub(st, 1))

        # ---------------- flush ----------------
        while slot_q:
            slot_q.popleft()()

        _DEBUG_TILES.update({
            'qT0': kq_tiles.get(('q', 0)), 'kT0': kq_tiles.get(('k', 0)),
            'vh0': vhat[0], 'at0': attn_outT.get(0), 'at1': attn_outT.get(1),
            'at2': attn_outT.get(2), 'at3': attn_outT.get(3),
        })


_NC_CACHE = None
_last_in_maps = None
_DEBUG_TILES = {}


def _get_nc():
    global _NC_CACHE
    if _NC_CACHE is None:
        _NC_CACHE = build()
    return _NC_CACHE


def kernel(Q, K, V, W_Q, b_Q, W_K, b_K, W_V, b_V, W_O, b_O):
    global _last_in_maps
    Q = np.asarray(Q, dtype=np.float32)
    K = np.asarray(K, dtype=np.float32)
    V = np.asarray(V, dtype=np.float32)
    nc = _get_nc()

    XqTs = [prep(Q[b].T, PROJ_DT) for b in range(B)]
    XkTs = [prep(K[b].T, PROJ_DT) for b in range(B)]
    XvTs = [prep(V[b].T, PROJ_DT) for b in range(B)]
    Wqs = [prep(np.asarray(W_Q)[:, hg * C:(hg + 1) * C], PROJ_DT)
           for hg in range(2)]
    Wks = [prep(np.asarray(W_K)[:, hg * C:(hg + 1) * C], PROJ_DT)
           for hg in range(2)]
    Wvs = [prep(np.asarray(W_V)[:, hg * C:(hg + 1) * C], PROJ_DT)
           for hg in range(2)]
    Wos = [prep(np.asarray(W_O)[hg * C:(hg + 1) * C, :], OUT_DT)
           for hg in range(2)]
    bqs = [np.ascontiguousarray(np.asarray(b_Q, dtype=np.float32)[hg * C:(hg + 1) * C])
           for hg in range(2)]
    bks = [np.ascontiguousarray(np.asarray(b_K, dtype=np.float32)[hg * C:(hg + 1) * C])
           for hg in range(2)]
    bvs = [np.ascontiguousarray(np.asarray(b_V, dtype=np.float32)[hg * C:(hg + 1) * C])
           for hg in range(2)]

    in_maps = []
    for c in range(N_CORES):
        b, hg = c // 2, c % 2
        in_maps.append({
            "XqT": XqTs[b], "XkT": XkTs[b], "XvT": XvTs[b],
            "Wq": Wqs[hg], "Wk": Wks[hg], "Wv": Wvs[hg], "Wo": Wos[hg],
            "bq": bqs[hg], "bk": bks[hg], "bv": bvs[hg],
        })
    _last_in_maps = in_maps
    res = run_bass_kernel_spmd(nc, in_maps, list(range(N_CORES)))
    out = np.empty((B, S, D), dtype=np.float32)
    bO = np.asarray(b_O, dtype=np.float32)
    for b in range(B):
        out[b] = res.results[2 * b]["OP"] + res.results[2 * b + 1]["OP"] + bO
    return out
